# revision 1
# baseline (speedup 1.0000x reference)
"""KitNET anomaly-detection ensemble (25 tiny tied-weight autoencoders) on 8 Trainium2 cores.

Strategy:
  - Data-parallel over batch: each of the 8 cores processes B/8 = 16384 samples.
  - The per-AE feature gather (x[:, idx]) is folded into block-structured dense
    matrices built on the host from idx (a partition of the 400 features):
        Wenc [400, 300]  : encode   pre_h = x @ Wenc  (+hb)
        Wdec [300, 400]  : decode   rec   = sigmoid(h) @ Wdec (+vb), in NATURAL
                           feature order so err = x - rec aligns elementwise
        G    [400, 25]   : per-AE group sums of err^2 as a matmul
  - On-chip layout is feature-major ("transposed"): x tiles are cast to bf16 and
    transposed via PE transpose-mode so the contraction dim (features / hidden)
    lives on partitions; sigmoid biases ride the ACT per-partition bias port.
  - Matmul operands are bf16 (full-rate PE); PSUM accumulation, biases, the
    per-AE sums S, sqrt and the final reduction are fp32.
  - sqrt(mean+eps) is phase-split to the end (single ACT table switch), final
    sum over the 25 AEs is a ones-block fp32 matmul.
"""

import sys

for _p in ("/opt/trn_rl_repo", "/opt/pypackages"):
    if _p not in sys.path:
        sys.path.append(_p)

import numpy as np

B = 131072
F = 400          # features
N_AE = 25
KF = 16          # features per AE
H = 12           # hidden per AE
EPS = 1e-6
N_CORES = 8
BC = B // N_CORES    # 16384 samples per core
NB = 512             # batch tile (matmul moving free dim)
NT = BC // NB        # 32 tiles per core
NH = N_AE * H        # 300 flattened hidden

_F_CH = (128, 128, 128, 16)     # feature chunks (contraction for encode / G)
_H_CH = (128, 128, 44)          # hidden chunks  (contraction for decode)
_FT_CH = (128, 128, 128, 16)    # feature tiles  (decode output partitions)

_NC_CACHE = {}


def _build_nc():
    import concourse.tile as tile
    from concourse import bacc, mybir

    f32 = mybir.dt.float32
    bf16 = mybir.dt.bfloat16
    AF = mybir.ActivationFunctionType

    nc = bacc.Bacc()

    x_d = nc.declare_dram_parameter("x", [BC, F], f32, isOutput=False)
    wenc_d = nc.declare_dram_parameter("wenc", [4, 128, NH], bf16, isOutput=False)
    wdec_d = nc.declare_dram_parameter("wdec", [3, 128, F], bf16, isOutput=False)
    g_d = nc.declare_dram_parameter("gmat", [4, 128, 32], bf16, isOutput=False)
    hb_d = nc.declare_dram_parameter("hbm", [128, 3], f32, isOutput=False)
    vb_d = nc.declare_dram_parameter("vbm", [128, 4], f32, isOutput=False)
    id_d = nc.declare_dram_parameter("ident", [128, 128], bf16, isOutput=False)
    y_d = nc.declare_dram_parameter("y", [BC], f32, isOutput=True)

    with tile.TileContext(nc) as tc:
        with (
            tc.tile_pool(name="singles", bufs=1) as singles,
            tc.tile_pool(name="xnat", bufs=3) as xnat_p,
            tc.tile_pool(name="xb", bufs=2) as xb_p,
            tc.tile_pool(name="xt", bufs=2) as xt_p,
            tc.tile_pool(name="ht", bufs=2) as ht_p,
            tc.tile_pool(name="rec", bufs=2) as rec_p,
            tc.tile_pool(name="xtp", bufs=2, space="PSUM") as xtp_p,
            tc.tile_pool(name="encp", bufs=2, space="PSUM") as encp_p,
            tc.tile_pool(name="decp", bufs=2, space="PSUM") as decp_p,
            tc.tile_pool(name="sp", bufs=2, space="PSUM") as sp_p,
        ):
            # --- constants ---
            ident = singles.tile([128, 128], bf16)
            nc.sync.dma_start(out=ident, in_=id_d[:, :])
            wenc_sb = singles.tile([128, 4, NH], bf16)
            nc.sync.dma_start(
                out=wenc_sb, in_=wenc_d[:, :, :].rearrange("c p n -> p c n")
            )
            wdec_sb = singles.tile([128, 3, F], bf16)
            nc.sync.dma_start(
                out=wdec_sb, in_=wdec_d[:, :, :].rearrange("c p n -> p c n")
            )
            g_sb = singles.tile([128, 4, 32], bf16)
            nc.sync.dma_start(out=g_sb, in_=g_d[:, :, :].rearrange("c p n -> p c n"))
            hb_sb = singles.tile([128, 3], f32)
            nc.sync.dma_start(out=hb_sb, in_=hb_d[:, :])
            vb_sb = singles.tile([128, 4], f32)
            nc.sync.dma_start(out=vb_sb, in_=vb_d[:, :])
            # per-AE squared-error sums for the whole core, 4 tiles stacked on
            # partitions (32-aligned): sall[32*(t%4) + a, t//4, i]
            sall = singles.tile([128, NT // 4, NB], f32)

            x_ap = x_d[:, :]

            for t in range(NT):
                # ---- load 512 samples: [128p, 4sb, 400f], b = t*512 + sb*128 + p
                xn = xnat_p.tile([128, 4, F], f32, tag="xn")
                nc.sync.dma_start(
                    out=xn,
                    in_=x_ap[t * NB:(t + 1) * NB, :].rearrange(
                        "(s p) f -> p s f", p=128
                    ),
                )
                xnb = xb_p.tile([128, 4, F], bf16, tag="xnb")
                nc.vector.tensor_copy(out=xnb, in_=xn)

                # ---- transpose to feature-major xt[f, fc, i], bf16
                xt = xt_p.tile([128, 4, NB], bf16, tag="xt")
                for fc, fw in enumerate(_F_CH):
                    pxt = xtp_p.tile([fw, NB], bf16, tag="xtp")
                    for sb in range(4):
                        nc.tensor.transpose(
                            pxt[:, sb * 128:(sb + 1) * 128],
                            xnb[:, sb, fc * 128:fc * 128 + fw],
                            ident,
                        )
                    nc.vector.tensor_copy(out=xt[0:fw, fc, :], in_=pxt)

                # ---- encode: hT = sigmoid(Wenc^T @ xT + hb)
                ht = ht_p.tile([128, 3, NB], bf16, tag="ht")
                for mt, mw in enumerate(_H_CH):
                    pe_ = encp_p.tile([mw, NB], f32, tag="encp")
                    for kc, kw in enumerate(_F_CH):
                        nc.tensor.matmul(
                            pe_,
                            lhsT=wenc_sb[0:kw, kc, mt * 128:mt * 128 + mw],
                            rhs=xt[0:kw, kc, :],
                            start=(kc == 0),
                            stop=(kc == 3),
                        )
                    nc.scalar.activation(
                        out=ht[0:mw, mt, :],
                        in_=pe_,
                        func=AF.Sigmoid,
                        bias=hb_sb[0:mw, mt:mt + 1],
                    )

                # ---- decode: recT = sigmoid(Wdec^T @ hT + vb), natural f order
                rec = rec_p.tile([128, 4, NB], bf16, tag="rec")
                for ft, fw in enumerate(_FT_CH):
                    pd = decp_p.tile([fw, NB], f32, tag="decp")
                    for hc, hw in enumerate(_H_CH):
                        nc.tensor.matmul(
                            pd,
                            lhsT=wdec_sb[0:hw, hc, ft * 128:ft * 128 + fw],
                            rhs=ht[0:hw, hc, :],
                            start=(hc == 0),
                            stop=(hc == 2),
                        )
                    nc.scalar.activation(
                        out=rec[0:fw, ft, :],
                        in_=pd,
                        func=AF.Sigmoid,
                        bias=vb_sb[0:fw, ft:ft + 1],
                    )

                # ---- err^2 (in place in rec)
                for ft, fw in enumerate(_FT_CH):
                    nc.vector.tensor_sub(
                        rec[0:fw, ft, :], xt[0:fw, ft, :], rec[0:fw, ft, :]
                    )
                    nc.scalar.activation(
                        out=rec[0:fw, ft, :], in_=rec[0:fw, ft, :], func=AF.Square
                    )

                # ---- per-AE sums: S[32*(t%4) + a, i] += G^T @ err2
                # (G is padded to 32 columns of which 25-31 are zero, so the
                # full 32-partition stripe is written — no PSUM garbage.)
                g = t % 4
                if g == 0:
                    ps4 = sp_p.tile([128, NB], f32, tag="sp")
                for kc, kw in enumerate(_F_CH):
                    nc.tensor.matmul(
                        ps4[32 * g:32 * (g + 1), :],
                        lhsT=g_sb[0:kw, kc, :],
                        rhs=rec[0:kw, kc, :],
                        start=(kc == 0),
                        stop=(kc == 3),
                        tile_position=(0, 32 * g),
                    )
                if g == 3:
                    nc.vector.tensor_copy(out=sall[:, t // 4, :], in_=ps4)

            # ---- phase B: rmse = sqrt(S/16 + eps); y = sum over AEs
            eps_sb = singles.tile([128, 1], f32)
            nc.vector.memset(eps_sb, EPS)
            nc.scalar.activation(
                out=sall, in_=sall, func=AF.Sqrt, bias=eps_sb, scale=1.0 / KF
            )
            ones4 = singles.tile([128, 4], f32)
            nc.gpsimd.memset(ones4, 0.0)
            for g in range(4):
                nc.gpsimd.memset(ones4[32 * g:32 * g + N_AE, g:g + 1], 1.0)
            ybuf = singles.tile([4, NT // 4, NB], f32)
            for j in range(NT // 4):
                py = sp_p.tile([4, NB], f32, tag="sp")
                nc.tensor.matmul(
                    py,
                    lhsT=ones4,
                    rhs=sall[:, j, :],
                    start=True,
                    stop=True,
                )
                if j % 2 == 0:
                    nc.vector.tensor_copy(out=ybuf[:, j, :], in_=py)
                else:
                    nc.scalar.copy(out=ybuf[:, j, :], in_=py)
            # y[b], b = t*NB + i, t = 4j + g  ->  y view [g, j, i]
            y_ap = y_d[:].rearrange("(j g i) -> g j i", g=4, i=NB)
            nc.sync.dma_start(out=y_ap, in_=ybuf)

    nc.compile()
    return nc


def _host_mats(W, hb, vb, idx):
    import ml_dtypes

    bf16 = ml_dtypes.bfloat16
    W = np.asarray(W, np.float32)
    hb = np.asarray(hb, np.float32)
    vb = np.asarray(vb, np.float32)
    idx = np.asarray(idx)

    wenc = np.zeros((512, NH), np.float32)
    wdec = np.zeros((384, F), np.float32)
    gmat = np.zeros((512, 32), np.float32)
    vb_nat = np.zeros((512,), np.float32)
    for a in range(N_AE):
        for k in range(KF):
            f = idx[a, k]
            wenc[f, a * H:(a + 1) * H] = W[a, k, :]
            wdec[a * H:(a + 1) * H, f] = W[a, k, :]
            vb_nat[f] = vb[a, k]
            gmat[f, a] = 1.0

    hb_flat = np.zeros((384,), np.float32)
    hb_flat[:NH] = hb.reshape(-1)
    hbm = np.zeros((128, 3), np.float32)
    for c in range(3):
        hbm[:, c] = hb_flat[c * 128:(c + 1) * 128]
    vbm = np.zeros((128, 4), np.float32)
    for c in range(4):
        vbm[:, c] = vb_nat[c * 128:(c + 1) * 128]

    return {
        "wenc": np.ascontiguousarray(wenc.reshape(4, 128, NH).astype(bf16)),
        "wdec": np.ascontiguousarray(wdec.reshape(3, 128, F).astype(bf16)),
        "gmat": np.ascontiguousarray(gmat.reshape(4, 128, 32).astype(bf16)),
        "hbm": hbm,
        "vbm": vbm,
        "ident": np.eye(128, dtype=np.float32).astype(bf16),
    }


def _get_nc():
    if "nc" not in _NC_CACHE:
        _NC_CACHE["nc"] = _build_nc()
    return _NC_CACHE["nc"]


def _run(x, W, hb, vb, idx, trace=False):
    from concourse.bass_utils import run_bass_kernel_spmd

    x = np.ascontiguousarray(np.asarray(x, np.float32))
    consts = _host_mats(W, hb, vb, idx)
    in_maps = [
        {"x": x[c * BC:(c + 1) * BC], **consts} for c in range(N_CORES)
    ]
    nc = _get_nc()
    res = run_bass_kernel_spmd(nc, in_maps, list(range(N_CORES)), trace=trace)
    y = np.concatenate([res.results[c]["y"] for c in range(N_CORES)])
    return y, res


def kernel(x, W, hb, vb, idx):
    y, _ = _run(x, W, hb, vb, idx)
    return y

